# revision 19
# baseline (speedup 1.0000x reference)
"""DIN attention layer kernel for Trainium2 (8 NeuronCores, data-parallel over batch).

Reference math:
  x  = concat([q, ub, q-ub, q*ub], -1)             # [B,T,144]
  h1 = sigmoid(x @ W1 + b1)                        # [B,T,80]
  h2 = sigmoid(h1 @ W2 + b2)                       # [B,T,40]
  s  = h2 @ W3 + b3                                # [B,T,1]
  w  = softmax(s.T * mask)                         # [B,1,T]  (multiplicative mask)
  out = w @ ub                                     # [B,1,36]

Host-side algebraic folds:
  1) x @ W1 = ub @ (Wb-Wc) + (q*ub) @ Wd + q @ (Wa+Wc); q is per-batch, so fold
     into per-batch weights Waug_b = [(Wb-Wc) + diag(q_b) Wd ; q_b(Wa+Wc)+b1]
     ([37,80]) and augment ub with a ones column -> single K=37 matmul.
  2) sigmoid(x) = 0.5 + 0.5*tanh(x/2); tanh and exp share one ACT table set,
     so the device computes t = tanh(pre/2) and the 0.5/0.5 affine is folded
     into the next layer's weights/biases.

Device strategy (bf16, 388us vs 1156us fp32 baseline):
  - Host pre-packs every DRAM array in the exact SBUF layout so each load is
    one contiguous DMA (the fp32 baseline spent ~775us of SP sequencer time
    generating gather descriptors).  All big loads use 128-partition tiles:
    descriptors of <=64-partition loads all execute on ONE DMA engine
    (22.5 GB/s) while 128-partition loads spread across all 16 engines —
    this alone was worth ~140us.  ubt/waug stack two 32-batch blocks at
    partition bases 0/64 (matmul operands may sit at base 64).
  - ubaug ships twice: transposed (feeding mm1 directly, no on-chip
    transposes) and batch-aligned natural tiles (feeding the weighted sum).
  - All matmuls bf16 (psum fp32): mm1 200 cols/batch, mm2 400 cols per 4
    batches, mm3 computes 4 batches per 400-col stream via a [104,2]
    two-block W3, then a [98,8] selection matmul compacts the scattered
    psum partitions into dense rows (batch = 16*(p//32) + 2*(p%32) + hc) —
    partition moves are only possible on PE or DMA, and per-descriptor DMA
    was the previous bottleneck.  Softmax runs on that psum layout
    directly; softmax weights are PE-transposed for the data-stationary
    weighted sum (ub tile as lhsT, weight columns as rhs, 2 matmuls/batch).
  - Work is software-pipelined: phase p's softmax + weighted sum interleave
    into phase p+1's MLP groups so PE/ACT/DVE overlap (PE ~80% busy).
  Pitfalls baked into the structure: psum matmul accumulation chains must
  not interleave with other matmuls (corruption); SBUF APs may cross
  partitions only in the first dim (offset = partition*pitch + col); the
  AP balancer must not be allowed to merge a partition-crossing dim with
  an inner dim (keep them non-mergeable or split calls).
"""

from contextlib import ExitStack

import numpy as np
import ml_dtypes

import concourse.bass as bass
import concourse.bacc as bacc
import concourse.tile as tile
from concourse import mybir
from concourse.bass_utils import run_bass_kernel_spmd

DEBUG_TAPS = False

B, T, E = 4096, 200, 36
N_CORES = 8
BC = B // N_CORES          # batches per core (512)
PB = 64                    # batches per phase
PH = BC // PB              # phases (8)
RP = PB * T                # rows per phase (12800)
F32 = mybir.dt.float32
BF16 = mybir.dt.bfloat16
AF = mybir.ActivationFunctionType
ALU = mybir.AluOpType
BF_NP = ml_dtypes.bfloat16


def dap(t, offset, dims):
    return bass.AP(tensor=t.tensor, offset=t.offset + offset, ap=dims)


def build_module():
    nc = bacc.Bacc(
        "TRN2", target_bir_lowering=False, debug=False,
        enable_asserts=False, num_devices=N_CORES,
    )

    # host-prepacked inputs (layouts match SBUF tiles; all DMAs are contiguous)
    ubt_d = nc.dram_tensor("ubt", [PH, 128, RP // 2], BF16,
                           kind="ExternalInput").ap()
    natb_d = nc.dram_tensor("natb", [PH, 128, PB * 2 * 37], BF16,
                            kind="ExternalInput").ap()
    waugt_d = nc.dram_tensor("waugt", [PH, 128, PB * 40], BF16,
                             kind="ExternalInput").ap()
    w2p_d = nc.dram_tensor("w2p", [80, 64], BF16, kind="ExternalInput").ap()
    w3o_d = nc.dram_tensor("w3o", [104, 32], BF16, kind="ExternalInput").ap()
    b2c_d = nc.dram_tensor("b2c", [128, 1], F32, kind="ExternalInput").ap()
    b3c_d = nc.dram_tensor("b3c", [104, 1], F32, kind="ExternalInput").ap()
    lens_d = nc.dram_tensor("lens", [104, 2 * PH], F32,
                            kind="ExternalInput").ap()
    out_d = nc.dram_tensor("out", [BC, 37], F32, kind="ExternalOutput").ap()
    if DEBUG_TAPS:
        sc_dram = nc.dram_tensor("sc_scratch", [BC * T], F32,
                                 kind="ExternalOutput").ap()
        w_dbg = nc.dram_tensor("w_dbg", [PH, 104, 400], F32,
                               kind="ExternalOutput").ap()
        h1_dbg = nc.dram_tensor("h1_dbg", [80, 800], BF16,
                                kind="ExternalOutput").ap()
        h2_dbg = nc.dram_tensor("h2_dbg", [128, 400], BF16,
                                kind="ExternalOutput").ap()
        mv_dbg = nc.dram_tensor("mv_dbg", [37, 64], F32,
                                kind="ExternalOutput").ap()

    ident104_d = nc.inline_tensor(np.eye(104, dtype=np.float32),
                                  name="ident104").ap()
    identf_d = nc.inline_tensor(np.eye(37, dtype=np.float32), name="identf").ap()
    iota_d = nc.inline_tensor(
        np.broadcast_to(np.tile(np.arange(T, dtype=np.float32), 2),
                        (104, 2 * T)).copy(),
        name="iotat").ap()

    with tile.TileContext(nc) as tc, ExitStack() as es:
        cpool = es.enter_context(tc.tile_pool(name="consts", bufs=1))
        ubtp = es.enter_context(tc.tile_pool(name="ubtp", bufs=2))
        natp = es.enter_context(tc.tile_pool(name="natp", bufs=3))
        waugp = es.enter_context(tc.tile_pool(name="waugp", bufs=2))
        h1p = es.enter_context(tc.tile_pool(name="h1p", bufs=3))
        h2p = es.enter_context(tc.tile_pool(name="h2p", bufs=6))
        smp = es.enter_context(tc.tile_pool(name="smp", bufs=2))
        wtp = es.enter_context(tc.tile_pool(name="wtp", bufs=2))
        mvsp = es.enter_context(tc.tile_pool(name="mvsp", bufs=2))
        otp = es.enter_context(tc.tile_pool(name="otp", bufs=2))
        m1p = es.enter_context(tc.tile_pool(name="m1p", bufs=3, space="PSUM"))
        m2p = es.enter_context(tc.tile_pool(name="m2p", bufs=3, space="PSUM"))
        sc2p = es.enter_context(tc.tile_pool(name="sc2p", bufs=1, space="PSUM"))
        smps = es.enter_context(tc.tile_pool(name="smps", bufs=1, space="PSUM"))

        ident104 = cpool.tile([104, 104], F32)
        nc.sync.dma_start(out=ident104, in_=ident104_d)
        identf = cpool.tile([37, 37], F32)
        nc.sync.dma_start(out=identf, in_=identf_d)
        iota_t = cpool.tile([104, 2 * T], F32)
        nc.sync.dma_start(out=iota_t, in_=iota_d)
        w2_t = cpool.tile([80, 64], BF16)
        nc.sync.dma_start(out=w2_t, in_=w2p_d)
        w3o_t = cpool.tile([104, 4, 8], BF16)
        nc.sync.dma_start(out=w3o_t, in_=w3o_d)
        b2_t = cpool.tile([128, 1], F32)
        nc.sync.dma_start(out=b2_t, in_=b2c_d)
        b3_t = cpool.tile([104, 1], F32)
        nc.sync.dma_start(out=b3_t, in_=b3c_d)
        lensall_t = cpool.tile([104, 2 * PH], F32)
        nc.sync.dma_start(out=lensall_t, in_=lens_d)

        loaded = {}

        def emit_loads(ph):
            # ubt/waug have only 37 partitions, so their per-partition-line
            # descriptors are huge and serialize on one DMA engine; split
            # into column chunks alternated across the two HWDGE queues
            # ubt/waug ship as 128-partition tiles (two 32-batch blocks at
            # partition bases 0 and 64, zero-padded rows between): DMA
            # descriptors for 128-partition contiguous loads spread across
            # all 16 DMA engines, while <=64-partition loads serialize on one
            ubt_t = ubtp.tile([128, RP // 2], BF16, tag="ubt", name=f"ubt{ph}")
            nc.scalar.dma_start(
                out=ubt_t, in_=dap(ubt_d, ph * 128 * (RP // 2),
                                   [[RP // 2, 128], [1, RP // 2]]))
            nat_t = natp.tile([128, PB, 2, 37], BF16, tag="nat", name=f"nat{ph}")
            nc.scalar.dma_start(
                out=nat_t,
                in_=dap(natb_d, ph * 128 * PB * 2 * 37,
                        [[PB * 2 * 37, 128], [1, PB * 2 * 37]]))
            waug_t = waugp.tile([128, PB // 2, 80], BF16, tag="waug",
                                name=f"waug{ph}")
            nc.sync.dma_start(
                out=waug_t,
                in_=dap(waugt_d, ph * 128 * PB * 40,
                        [[PB * 40, 128], [1, PB * 40]]))
            loaded[ph] = (ubt_t, nat_t, waug_t, lensall_t[:, ph:ph + 1])

        def emit_wt_transposes(ph):
            """Transpose softmax weights of phase ph for the weighted sum."""
            wb = loaded[ph + 100]  # w_t [104, 2, 200]
            smt = smps.tile([128, 128], F32, tag="sm", name=f"smt{ph}")
            wT = {}
            for hc in range(2):
                nc.tensor.transpose(
                    smt[0:128, 0:104], wb[:, hc, 0:128], ident104)
                wT0 = wtp.tile([128, 104], BF16, tag=f"wt0{hc}",
                               name=f"wt0{hc}_{ph}")
                nc.vector.tensor_copy(out=wT0, in_=smt[0:128, 0:104])
                nc.tensor.transpose(
                    smt[0:72, 0:104], wb[:, hc, 128:200], ident104)
                wT1 = wtp.tile([72, 104], BF16, tag=f"wt1{hc}",
                               name=f"wt1{hc}_{ph}")
                nc.vector.tensor_copy(out=wT1, in_=smt[0:72, 0:104])
                wT[hc] = (wT0, wT1)
            loaded[ph + 200] = (smt, wT)

        def emit_mv(ph, b0, b1):
            """Weighted-sum matmuls for batches [b0, b1) of phase ph."""
            nat_t = loaded[ph][1]
            smt, wT = loaded[ph + 200]
            for b in range(b0, b1):
                hc = b % 2
                col = 32 * (b // 16) + (b % 16) // 2
                wT0, wT1 = wT[hc]
                nc.tensor.matmul(
                    smt[0:37, b:b + 1], nat_t[:, b, 0, :],
                    wT0[:, col:col + 1], start=True, stop=False)
                nc.tensor.matmul(
                    smt[0:37, b:b + 1], nat_t[0:72, b, 1, :],
                    wT1[:, col:col + 1], start=False, stop=True)

        def emit_out(ph):
            smt, _ = loaded[ph + 200]
            mvs = mvsp.tile([37, 64], F32, tag="mvs", name=f"mvs{ph}")
            nc.vector.tensor_copy(out=mvs, in_=smt[0:37, 0:64])
            if DEBUG_TAPS and ph == 0:
                nc.sync.dma_start(out=mv_dbg, in_=mvs)
            nc.tensor.transpose(smt[0:64, 64:101], mvs, identf)
            ot = otp.tile([64, 37], F32, tag="ot", name=f"ot{ph}")
            nc.vector.tensor_copy(out=ot, in_=smt[0:64, 64:101])
            nc.sync.dma_start(
                out=dap(out_d, 37 * PB * ph, [[37, 64], [1, 37]]),
                in_=ot)

        def emit_softmax(ph, sc_t):
            # sc_t: [104, 2, 200] psum; batch = 16*(p//32) + 2*(p%32) + hc
            mask_t = smp.tile([104, 2, T], F32, tag="mask", name=f"mask{ph}")
            for hc in range(2):
                nc.vector.tensor_scalar(
                    out=mask_t[:, hc, :], in0=iota_t[:, 0:200],
                    scalar1=lensall_t[:, 2 * ph + hc:2 * ph + hc + 1],
                    scalar2=None, op0=ALU.is_lt)
            masked = smp.tile([104, 2, T], F32, tag="masked", name=f"masked{ph}")
            nc.vector.scalar_tensor_tensor(
                out=masked, in0=sc_t.rearrange("p (u c) -> p u c", u=2),
                scalar=b3_t, in1=mask_t, op0=ALU.add, op1=ALU.mult)
            negmax = smp.tile([104, 2], F32, tag="negmax", name=f"negmax{ph}")
            nc.vector.tensor_reduce(
                out=negmax, in_=masked, axis=mybir.AxisListType.X,
                op=ALU.max, negate=True)
            ew = smp.tile([104, 2, T], F32, tag="ew", name=f"ew{ph}")
            sumexp = smp.tile([104, 2], F32, tag="sumexp", name=f"sumexp{ph}")
            for hc in range(2):
                nc.scalar.activation(
                    out=ew[:, hc, :], in_=masked[:, hc, :], func=AF.Exp,
                    bias=negmax[:, hc:hc + 1],
                    accum_out=sumexp[:, hc:hc + 1])
            rz = smp.tile([104, 2], F32, tag="rz", name=f"rz{ph}")
            nc.vector.reciprocal(rz, sumexp)
            w_t = smp.tile([104, 2, T], F32, tag="wt", name=f"wt{ph}")
            for hc in range(2):
                nc.vector.tensor_scalar_mul(
                    w_t[:, hc, :], ew[:, hc, :], rz[:, hc:hc + 1])
            if DEBUG_TAPS:
                nc.sync.dma_start(
                    out=dap(w_dbg, 2 * T * 104 * ph, [[400, 104], [1, 400]]),
                    in_=w_t)
            loaded[ph + 100] = w_t

        emit_loads(0)
        for ph in range(PH):
            if ph + 1 < PH:
                emit_loads(ph + 1)
            ubt_t, nat_t, waug_t, lens_t = loaded[ph]
            prev = ph - 1 if ph > 0 else None

            m1_tiles = {}
            h1_tiles = {}
            h2_tiles = {}
            sc_t_phase = [sc2p.tile([104, 400], F32, tag="sc2",
                                    name=f"sc2_{ph}")]

            # 16 groups of 4 batches + 3 drain iterations, software-pipelined:
            # PE order per iter: mm1(g), [tail work of prev phase], mm2(g-1),
            # mm3(g-2).  ACT order: h1(g), h2(g-1).
            for g in range(19):
                if g < 16:
                    h1_t = h1p.tile([80, 2, 400], BF16, tag="h1", name=f"h1_{ph}_{g}")
                    for u in range(2):
                        m1_ps = m1p.tile([80, 512], F32, tag="m1",
                                         name=f"m1_{ph}_{g}_{u}")
                        for j in range(2):
                            b = 4 * g + 2 * u + j
                            base = 64 * (b // 32)
                            bl = b % 32
                            nc.tensor.matmul(
                                m1_ps[0:80, 200 * j:200 * j + 200],
                                waug_t[base:base + 37, bl, :],
                                ubt_t[base:base + 37,
                                      200 * bl:200 * bl + 200],
                                start=True, stop=True)
                        nc.scalar.activation(
                            out=h1_t[:, u, :], in_=m1_ps[0:80, 0:400],
                            func=AF.Tanh, scale=0.5)
                    h1_tiles[g] = h1_t
                    if DEBUG_TAPS and ph == 0 and g == 0:
                        nc.sync.dma_start(
                            out=dap(h1_dbg, 0, [[800, 80], [1, 800]]),
                            in_=dap(h1_t, 0, [[800, 80], [1, 800]]))

                if prev is not None:
                    if g == 6:
                        emit_wt_transposes(prev)
                    if 7 <= g <= 16:
                        b0 = 7 * (g - 7)
                        b1 = min(64, 7 * (g - 6))
                        emit_mv(prev, b0, b1)
                    if g == 17:
                        emit_out(prev)

                if 1 <= g <= 16:
                    g1 = g - 1
                    h1_t = h1_tiles.pop(g1)
                    m2_ps = m2p.tile([128, 400], F32, tag="m2", name=f"m2_{ph}_{g1}")
                    for u in range(2):
                        nc.tensor.matmul(
                            m2_ps[64 * u:64 * u + 64, 0:400], w2_t,
                            h1_t[:, u, :], start=True, stop=True)
                    h2_t = h2p.tile([128, 400], BF16, tag="h2", name=f"h2_{ph}_{g1}")
                    nc.scalar.activation(
                        out=h2_t, in_=m2_ps, func=AF.Tanh, bias=b2_t, scale=0.5)
                    h2_tiles[g1] = h2_t
                    m1_tiles.pop(g1, None)
                    if DEBUG_TAPS and ph == 0 and g1 == 0:
                        nc.sync.dma_start(
                            out=dap(h2_dbg, 0, [[400, 128], [1, 400]]),
                            in_=dap(h2_t, 0, [[400, 128], [1, 400]]))

                if 3 <= g <= 18:
                    g2 = g - 3
                    g16 = g2 // 4
                    if g2 % 4 == 3:
                        # w3o_t[:, q, :] is [104, 8] with w3 in col 2q+hc for
                        # the hc row-block; 4 accumulating matmuls produce the
                        # dense batch = 16*(p//32) + 2*(p%32) + hc layout in
                        # sc2 directly (no sel compaction / bf16 recast).
                        # Issued as an uninterrupted burst: an accumulation
                        # chain interleaved with other matmuls corrupts psum.
                        for qq in range(4):
                            h2_t = h2_tiles.pop(4 * g16 + qq)
                            nc.tensor.matmul(
                                sc_t_phase[0][32 * g16:32 * g16 + 8, 0:400],
                                w3o_t[:, qq, :], h2_t[0:104, 0:400],
                                start=(qq == 0), stop=(qq == 3),
                                tile_position=(0, 32 * g16))

            emit_softmax(ph, sc_t_phase[0])

        # tail: softmax-weighted sum for the last phase
        emit_wt_transposes(PH - 1)
        emit_mv(PH - 1, 0, 64)
        emit_out(PH - 1)

    nc.compile()
    return nc


def host_prep(query_ad, user_behavior, user_behavior_length,
              W1, b1, W2, b2, W3, b3):
    q = np.asarray(query_ad, dtype=np.float32)
    ub = np.asarray(user_behavior, dtype=np.float32)
    lens = np.asarray(user_behavior_length)
    W1 = np.asarray(W1, dtype=np.float32)
    b1 = np.asarray(b1, dtype=np.float32)
    W2 = np.asarray(W2, dtype=np.float32)
    b2 = np.asarray(b2, dtype=np.float32)
    W3 = np.asarray(W3, dtype=np.float32)
    b3 = np.asarray(b3, dtype=np.float32)
    nb = q.shape[0]

    Wa, Wb, Wc, Wd = W1[0:36], W1[36:72], W1[72:108], W1[108:144]
    waug = np.empty((nb, 37, 80), dtype=np.float32)
    waug[:, 0:36, :] = (Wb - Wc)[None, :, :] + q[:, :, None] * Wd[None, :, :]
    waug[:, 36, :] = q @ (Wa + Wc) + b1[None, :]

    ubaug = np.empty((nb, T, 37), dtype=np.float32)
    ubaug[:, :, 0:36] = ub
    ubaug[:, :, 36] = 1.0

    # sigmoid -> tanh fold: h = 0.5 + 0.5*t with t = tanh(pre/2)
    w2f = 0.5 * W2
    b2f = 0.5 * (b2 + 0.5 * W2.sum(axis=0))
    w3f = 0.5 * W3
    b3f = float(b3[0] + 0.5 * W3.sum())

    w2p = np.zeros((80, 64), dtype=np.float32)
    w2p[:, 0:40] = w2f
    w3o = np.zeros((104, 4, 8), dtype=np.float32)
    for q in range(4):
        w3o[0:40, q, 2 * q] = w3f[:, 0]
        w3o[64:104, q, 2 * q + 1] = w3f[:, 0]
    w3o = w3o.reshape(104, 32)
    b2c = np.zeros((128, 1), dtype=np.float32)
    b2c[0:40, 0] = b2f
    b2c[64:104, 0] = b2f
    b3c = np.full((104, 1), b3f, dtype=np.float32)

    w2p = w2p.astype(BF_NP)
    w3o = w3o.astype(BF_NP)

    n_cores = nb // BC
    in_maps = []
    for c in range(n_cores):
        sl = slice(BC * c, BC * (c + 1))
        ub_c = ubaug[sl]                                    # [512, 200, 37]
        # mm1 rhs: transposed rows, two 32-batch blocks stacked at
        # partition bases 0/64 -> [PH, 128, RP/2]
        ubt_r = ub_c.reshape(PH, 2, RP // 2, 37).transpose(0, 3, 1, 2)
        ubt = np.zeros((PH, 128, RP // 2), dtype=np.float32)
        ubt[:, 0:37] = ubt_r[:, :, 0]
        ubt[:, 64:101] = ubt_r[:, :, 1]
        ubt = ubt.astype(BF_NP)
        # weighted-sum lhsT: batch-aligned natural tiles [PH, 128, PB, 2, 37]
        pad = np.zeros((BC, 256, 37), dtype=np.float32)
        pad[:, 0:T] = ub_c
        natb = np.ascontiguousarray(
            pad.reshape(PH, PB, 2, 128, 37).transpose(0, 3, 1, 2, 4)
        ).reshape(PH, 128, PB * 2 * 37).astype(BF_NP)
        waug_r = waug[sl].reshape(PH, 2, PB // 2, 37, 80).transpose(0, 3, 1, 2, 4)
        waugt = np.zeros((PH, 128, PB // 2, 80), dtype=np.float32)
        waugt[:, 0:37] = waug_r[:, :, 0]
        waugt[:, 64:101] = waug_r[:, :, 1]
        waugt = waugt.reshape(PH, 128, PB * 40).astype(BF_NP)
        lensf = lens[sl].astype(np.float32)
        lensc = np.zeros((104, 2 * PH), dtype=np.float32)
        for p in range(104):
            if p % 32 < 8:
                for ph in range(PH):
                    for hc in range(2):
                        bb = 64 * ph + 16 * (p // 32) + 2 * (p % 32) + hc
                        lensc[p, 2 * ph + hc] = lensf[bb]
        in_maps.append({
            "ubt": ubt, "natb": natb, "waugt": waugt,
            "w2p": w2p, "w3o": w3o, "b2c": b2c, "b3c": b3c,
            "lens": lensc,
        })
    return in_maps


_NC_CACHE = {}


def get_module():
    if "nc" not in _NC_CACHE:
        _NC_CACHE["nc"] = build_module()
    return _NC_CACHE["nc"]


def kernel(query_ad, user_behavior, user_behavior_length,
           W1, b1, W2, b2, W3, b3, trace=False):
    nc = get_module()
    in_maps = host_prep(query_ad, user_behavior, user_behavior_length,
                        W1, b1, W2, b2, W3, b3)
    res = run_bass_kernel_spmd(nc, in_maps, core_ids=list(range(N_CORES)),
                               trace=trace)
    outs = [np.asarray(res.results[c]["out"])[:, 0:36] for c in range(N_CORES)]
    full = np.ascontiguousarray(np.concatenate(outs, axis=0)).reshape(B, 1, 36)
    kernel.last_results = res.results
    if trace:
        kernel.last_result = res
    return full

